# revision 39
# baseline (speedup 1.0000x reference)
"""Trainium2 Bass kernel for nn_MAB_17471926960685 (dense_transformer).

Sharding: token-parallel over N. Each of 8 cores takes a 256-token slice of
N (both batches); K/V are computed replicated from Y. No collectives.

Design:
- Host folds both encoding tables into ONE fused table
  EF = exp(add_enc/16) * mult_enc, streamed bf16 in DMA-optimal layout.
  Softmax denominator approximated by sum_k at (mult_enc perturbs it by
  ~0.05%). bk dropped (softmax shift-invariant), bv folded into the
  mix-stage row bias, LN gains folded into weights.
- Scores keys-major: QK matmuls, ScalarE exp with scale=1/16, EF multiply
  split DVE/GpSimd, AV matmul with [V|ones] stationary so the denominator
  rides as output partition 32; normalize via f32r broadcast matmul.
- fp8 (e4m3, x16-scaled weights) with DoubleRow matmuls on the
  score/attention-value paths only (K, V, Q-scores, mix); the residual Q
  and FFN stay bf16 for accuracy.
- All layouts host-pretransposed; consolidated blob DMAs.
"""

import math
import sys

import numpy as np
import ml_dtypes

sys.path.insert(0, "/opt/trn_rl_repo")

import concourse.bass as bass
import concourse.mybir as mybir
import concourse.tile as tile
from concourse import bacc
from concourse.masks import make_identity
from concourse.bass_utils import run_bass_kernel_spmd

B, N, D, H = 2, 2048, 256, 8
DS = D // H          # 32
NCORES = 8
NL = N // NCORES     # 256 tokens per core per batch
TOK = B * NL         # 512 tokens per core
NKT = N // 128       # 16 key tiles
EPS = 1e-5
F32 = mybir.dt.float32
F32R = mybir.dt.float32r
BF16 = mybir.dt.bfloat16
FP8 = mybir.dt.float8e4
AX = mybir.AluOpType
AF = mybir.ActivationFunctionType
DR = mybir.MatmulPerfMode.DoubleRow

# how many of the 4 EF-mult chunks per (h,b) go to GpSimd (rest on DVE)
POOL_EF_CHUNKS = 0


def build_kernel(gelu_af=AF.Gelu_apprx_tanh):
    nc = bacc.Bacc()
    P = {}
    P["EF"] = nc.declare_dram_parameter("EF", [H, 128, NKT * NL], BF16,
                                        isOutput=False)
    P["W8"] = nc.declare_dram_parameter("W8", [128, 2048], FP8, isOutput=False)
    P["YTb"] = nc.declare_dram_parameter("YTb", [128, 4 * N], FP8,
                                         isOutput=False)
    P["Wb"] = nc.declare_dram_parameter("Wb", [128, 6656], BF16,
                                        isOutput=False)
    for name, shape in [
        ("Xs", [B, NL, D]),
        ("bias", [128, 24]),
        ("bqmix", [D]),
    ]:
        P[name] = nc.declare_dram_parameter(name, shape, F32, isOutput=False)
    out_ext = nc.declare_dram_parameter("out", [B, NL, D], F32, isOutput=True)

    with tile.TileContext(nc) as tc:
        with tc.tile_pool(name="pp", bufs=1) as pp, \
             tc.tile_pool(name="efp", bufs=2) as efp, \
             tc.tile_pool(name="atp", bufs=2) as atp, \
             tc.tile_pool(name="ep", bufs=4) as ep, \
             tc.tile_pool(name="smp", bufs=3) as smp, \
             tc.tile_pool(name="psS", bufs=2, space="PSUM") as psS, \
             tc.tile_pool(name="psA", bufs=2, space="PSUM") as psA, \
             tc.tile_pool(name="psB", bufs=2, space="PSUM") as psB:

            def ppt(shape, dtype, nm):
                return pp.tile(shape, dtype, tag=nm, name=nm)

            # ---------- constants ----------
            id128 = ppt([128, 128], BF16, "id128")
            make_identity(nc, id128)
            eps_t = ppt([128, 1], F32, "eps_t")
            nc.vector.memset(eps_t, EPS)

            # ---------- consolidated input loads ----------
            w8 = ppt([128, 2048], FP8, "w8")
            nc.sync.dma_start(out=w8, in_=P["W8"][:, :])
            ytb = ppt([128, 4 * N], FP8, "ytb")
            nc.scalar.dma_start(out=ytb, in_=P["YTb"][:, :])
            x_ts = []
            for b in range(B):
                x_t = ppt([128, 2 * D], F32, f"x_t{b}")
                nc.sync.dma_start(
                    out=x_t.rearrange("p (s d) -> p s d", s=2),
                    in_=P["Xs"][b].rearrange("(s p) d -> p s d", p=128))
                x_ts.append(x_t)
            bias_t = ppt([128, 24], F32, "bias_t")
            nc.scalar.dma_start(out=bias_t, in_=P["bias"][:, :])
            wb = ppt([128, 6656], BF16, "wb")
            nc.sync.dma_start(out=wb, in_=P["Wb"][:, :])
            # bqmix broadcast row -> [128, D]
            bqmix_bc = ppt([128, D], F32, "bqmix_bc")
            ap = P["bqmix"][:].rearrange("(o d) -> o d", o=1)
            bap = bass.AP(tensor=ap.tensor, offset=ap.offset,
                          ap=[[0, 128], ap.ap[1]])
            nc.sync.dma_start(out=bqmix_bc, in_=bap)

            # fp8 blob slices as [K, 2, M] APs (DoubleRow; weights x16)
            def pair2(ap2d):
                return ap2d.rearrange("p (t c) -> p t c", t=2)

            WqT8 = pair2(w8[:, 0:512])            # [128, 2, 256]
            WkT8 = pair2(w8[:, 512:1024])
            WvT8 = pair2(w8[:, 1024:1536])
            WmixT8 = pair2(w8[:, 1536:2048])
            yT8 = [pair2(ytb[:, b * 2 * N:(b * 2 + 2) * N]) for b in range(B)]
            # bf16 blob slices
            wi0T = [wb[:, dd * 1024:(dd + 1) * 1024] for dd in range(2)]
            wi1T = [wb[:, 2048 + dd * 1024:2048 + (dd + 1) * 1024]
                    for dd in range(2)]
            woT = [wb[:, 4096 + m * D:4096 + (m + 1) * D] for m in range(8)]
            WqTb = [wb[:, 6144 + dd * D:6144 + (dd + 1) * D]
                    for dd in range(2)]
            bqc = [bias_t[0:64, gg:gg + 1] for gg in range(4)]
            b0c = [bias_t[:, 4 + m:5 + m] for m in range(8)]
            b1c = [bias_t[:, 12 + m:13 + m] for m in range(8)]

            # ---------- phase 1: LN0 (DVE; overlaps K/V on PE) ----------
            lnx = []
            for b in range(B):
                x_t = x_ts[b]
                for s in range(2):
                    xa = x_t[:, s * D:(s + 1) * D]
                    stats = smp.tile([128, 6], F32, tag="st", name="stats")
                    mv = smp.tile([128, 2], F32, tag="mv", name="mv")
                    nc.vector.bn_stats(out=stats, in_=xa)
                    nc.vector.bn_aggr(out=mv, in_=stats)
                    std = smp.tile([128, 1], F32, tag="std", name="std")
                    nc.scalar.activation(std, mv[:, 1:2], AF.Sqrt, bias=eps_t)
                    rstd = smp.tile([128, 1], F32, tag="rstd", name="rstd")
                    nc.vector.reciprocal(rstd, std)
                    o = ppt([128, D], BF16, f"lnx{b}{s}")
                    nc.vector.tensor_scalar(o, xa, mv[:, 0:1], rstd,
                                            AX.subtract, AX.mult)
                    lnx.append(o)

            # ---------- phase 2: K^T/V as chunk emitters ----------
            # Emitted partly up front, partly interleaved into the attention
            # loop as PE filler (keeps the PE busy while exp/EF-mult run).
            kT = [ppt([64, N], BF16, f"kT{b}{gg}")
                  for b in range(B) for gg in range(4)]
            vN33 = []
            for b in range(B):
                t = ppt([128, NKT * 264], BF16, f"vN33_{b}")
                nc.gpsimd.memset(
                    t.rearrange("p (k h c) -> p k h c",
                                k=NKT, h=8)[:, :, :, 32:33],
                    1.0)
                vN33.append(t)

            def emit_k_chunk(b, gg, ch):
                t = kT[b * 4 + gg]
                ps = psB.tile([128, TOK], F32, tag="b", name="ps_k")
                sl = slice(ch * 512, (ch + 1) * 512)
                nc.tensor.matmul(
                    ps[0:64, :], WkT8[:, :, gg * 64:(gg + 1) * 64],
                    yT8[b][:, :, sl], start=True, stop=True, perf_mode=DR)
                nc.scalar.mul(t[:, sl], ps[0:64, :], 1.0 / 16.0)

            def emit_v_chunk(b, ktp):
                ps = psB.tile([128, TOK], F32, tag="b", name="ps_v")
                for q in range(2):
                    kt = ktp * 2 + q
                    nc.tensor.matmul(
                        ps[:, q * D:(q + 1) * D],
                        yT8[b][:, :, kt * 128:(kt + 1) * 128],
                        WvT8, start=True, stop=True, perf_mode=DR)
                dst = vN33[b][:, ktp * 528:(ktp + 1) * 528].rearrange(
                    "p (k h c) -> p k h c", k=2, h=8)[:, :, :, 0:32]
                src = ps.rearrange("p (k h c) -> p k h c", k=2, h=8)
                nc.scalar.mul(dst, src, 1.0 / 16.0)

            # up-front: V(b0) + K(b0, gg0)
            for ktp in range(8):
                emit_v_chunk(0, ktp)
            for ch in range(4):
                emit_k_chunk(0, 0, ch)
            # filler queue, deadline-ordered for b-major item order
            fillers = []
            for gg in (1, 2, 3):
                fillers += [lambda gg=gg, ch=ch: emit_k_chunk(0, gg, ch)
                            for ch in range(4)]
            fillers += [lambda ch=ch: emit_k_chunk(1, 0, ch)
                        for ch in range(4)]
            fillers += [lambda ktp=ktp: emit_v_chunk(1, ktp)
                        for ktp in range(8)]
            for gg in (1, 2, 3):
                fillers += [lambda gg=gg, ch=ch: emit_k_chunk(1, gg, ch)
                            for ch in range(4)]
            FILL_PER_STEP = [3, 3, 3, 3, 3, 3, 3, 3, 2, 2, 2, 2, 2, 2, 0, 0]

            # transpose lnx -> lnxT (de-major): bf16 for qN, fp8 for scores
            lnxTb = ppt([128, 2 * TOK], BF16, "lnxTb")
            lnxT8 = ppt([128, 2 * TOK], FP8, "lnxT8")
            for dd in range(2):
                pt = psB.tile([128, TOK], F32, tag="b", name="pt_lnx")
                ptb = pt.bitcast(BF16)          # [128, 1024] bf16 view
                for tt in range(4):
                    nc.tensor.transpose(ptb[:, tt * 128:(tt + 1) * 128],
                                        lnx[tt][:, dd * 128:(dd + 1) * 128],
                                        id128)
                nc.vector.tensor_copy(lnxTb[:, dd * TOK:(dd + 1) * TOK],
                                      ptb[:, 0:TOK])
                nc.vector.tensor_copy(lnxT8[:, dd * TOK:(dd + 1) * TOK],
                                      ptb[:, 0:TOK])
            lnxT8p = pair2(lnxT8[:, :])

            # Q^T (de-major, fp8 DR) for scores: 4 tiles of 64 rows
            # (2 heads each; operand base partitions must be in {0,32,64})
            qsT = []
            for gg in range(4):
                ps = psB.tile([128, TOK], F32, tag="b", name="ps_q")
                nc.tensor.matmul(ps[0:64, :],
                                 WqT8[:, :, gg * 64:(gg + 1) * 64],
                                 lnxT8p, start=True, stop=True, perf_mode=DR)
                t = ppt([64, TOK], BF16, f"qsT{gg}")
                nc.vector.tensor_scalar(t, ps[0:64, :], 1.0 / 16.0, bqc[gg],
                                        AX.mult, AX.add)
                qsT.append(t)
            # Q residual (tok-major, bf16) + bqmix row
            qN = []
            for pair in range(2):
                ps = psB.tile([128, TOK], F32, tag="b", name="ps_qn")
                for q in range(2):
                    blk = pair * 2 + q
                    for dd in range(2):
                        nc.tensor.matmul(
                            ps[:, q * D:(q + 1) * D],
                            lnxTb[:, dd * TOK + blk * 128:
                                  dd * TOK + (blk + 1) * 128],
                            WqTb[dd], start=(dd == 0), stop=(dd == 1))
                for q in range(2):
                    t = ppt([128, D], F32, f"qN{pair * 2 + q}")
                    nc.vector.tensor_tensor(t, ps[:, q * D:(q + 1) * D],
                                            bqmix_bc, AX.add)
                    qN.append(t)

            hid, hr = [None] * 4, [None] * 4

            def emit_ln1(blk):
                stats = smp.tile([128, 6], F32, tag="st", name="stats1")
                mv = smp.tile([128, 2], F32, tag="mv", name="mv1")
                nc.vector.bn_stats(out=stats, in_=hid[blk])
                nc.vector.bn_aggr(out=mv, in_=stats)
                std = smp.tile([128, 1], F32, tag="std", name="std1")
                nc.scalar.activation(std, mv[:, 1:2], AF.Sqrt, bias=eps_t)
                rstd = smp.tile([128, 1], F32, tag="rstd", name="rstd1")
                nc.vector.reciprocal(rstd, std)
                t = ppt([128, D], BF16, f"hr{blk}")
                nc.vector.tensor_scalar(t, hid[blk], mv[:, 0:1], rstd,
                                        AX.subtract, AX.mult)
                hr[blk] = t

            def emit_mix(pair):
                ps = psB.tile([128, TOK], F32, tag="b", name="ps_mx")
                for q in range(2):
                    blk = pair * 2 + q
                    nc.tensor.matmul(
                        ps[:, q * D:(q + 1) * D],
                        mhT8p[:, :, blk * 128:(blk + 1) * 128],
                        WmixT8, start=True, stop=True, perf_mode=DR)
                for q in range(2):
                    blk = pair * 2 + q
                    t = ppt([128, D], F32, f"hid{blk}")
                    nc.vector.scalar_tensor_tensor(
                        t, ps[:, q * D:(q + 1) * D], 1.0 / 256.0, qN[blk],
                        AX.mult, AX.add)
                    hid[blk] = t
                for q in range(2):
                    emit_ln1(pair * 2 + q)

            # ---------- phase 3: attention ----------
            # 2-stage software pipeline: step i emits rcp(i-2),
            # QK/exp/EFmult(i), AV(i-1), bc/normalize(i-2)
            mhT8 = ppt([128, 2 * TOK], FP8, "mhT8")
            mhT8p = pair2(mhT8[:, :])

            def emit_av(item):
                at, h, b = item[0], item[2], item[3]
                psav = psA.tile([128, 512], F32, tag="av", name="psav")
                item[1] = psav
                for kt in range(NKT):
                    nc.tensor.matmul(
                        psav[0:33, 0:256],
                        vN33[b][:, kt * 264 + h * 33:kt * 264 + h * 33 + 33],
                        at[:, kt * 256:(kt + 1) * 256],
                        start=(kt == 0), stop=(kt == NKT - 1))

            def emit_rcp(item):
                psav = item[1]
                rcp = smp.tile([1, 256], F32, tag="rcp", name="rcp")
                item[4] = rcp
                nc.vector.reciprocal(rcp, psav[32:33, 0:256])
                nc.vector.tensor_scalar_mul(rcp, rcp, 16.0)
                rb = smp.tile([32, 256], F32, tag="rb", name="rb")
                item[5] = rb
                nc.gpsimd.partition_broadcast(rb, rcp)

            def emit_final(item):
                psav, h, b, rb = item[1], item[2], item[3], item[5]
                g, j = h // 4, h % 4
                nc.vector.tensor_tensor(
                    mhT8[j * 32:(j + 1) * 32,
                         g * TOK + b * 256:g * TOK + (b + 1) * 256],
                    psav[0:32, 0:256], rb, AX.mult)

            items = []
            fq = list(fillers)
            for b in range(B):
                for h in range(H):
                    gg, jj = h // 2, h % 2
                    i = len(items)
                    ef_t = efp.tile([128, NKT * NL], BF16, tag="ef",
                                    name="ef_t")
                    nc.sync.dma_start(out=ef_t, in_=P["EF"][h])
                    if i >= 2:
                        emit_rcp(items[i - 2])   # DVE: ahead of EF-mults
                    for _ in range(FILL_PER_STEP[i]):
                        if fq:
                            fq.pop(0)()
                    at = atp.tile([128, NKT * NL], BF16, tag="at", name="at")
                    for c in range(4):
                        ps = psS.tile([128, 1024], F32, tag="s", name="ps_s")
                        for q in range(4):
                            kt = c * 4 + q
                            nc.tensor.matmul(
                                ps[:, q * 256:(q + 1) * 256],
                                kT[b * 4 + gg][jj * 32:(jj + 1) * 32,
                                               kt * 128:(kt + 1) * 128],
                                qsT[gg][jj * 32:(jj + 1) * 32,
                                        b * 256:(b + 1) * 256],
                                start=True, stop=True)
                        e_t = ep.tile([128, 1024], BF16, tag="e", name="e_t")
                        nc.scalar.activation(e_t, ps, AF.Exp, scale=1.0 / 16.0)
                        eng = nc.gpsimd if c < POOL_EF_CHUNKS else nc.vector
                        eng.tensor_tensor(
                            at[:, c * 1024:(c + 1) * 1024], e_t,
                            ef_t[:, c * 1024:(c + 1) * 1024], AX.mult)
                    items.append([at, None, h, b, None, None])
                    if i >= 1:
                        emit_av(items[i - 1])
                    if i >= 2:
                        emit_final(items[i - 2])
                    if i == 11:
                        emit_mix(0)   # batch-0 mix/LN1 during b1 attention
            assert not fq
            emit_av(items[-1])
            emit_rcp(items[-2])
            emit_final(items[-2])
            emit_rcp(items[-1])
            emit_final(items[-1])

            # ---------- phase 4: mix (fp8 DR) + residual + LN1 ----------
            emit_mix(1)
            hrT = []
            for dd in range(2):
                pt = psA.tile([128, 512], F32, tag="av", name="pt_hr")
                ptb = pt.bitcast(BF16)
                for tt in range(4):
                    nc.tensor.transpose(ptb[:, tt * 128:(tt + 1) * 128],
                                        hr[tt][:, dd * 128:(dd + 1) * 128],
                                        id128)
                t = ppt([128, TOK], BF16, f"hrT{dd}")
                nc.vector.tensor_copy(t, ptb[:, 0:TOK])
                hrT.append(t)

            # ---------- phase 5: FFN (bf16) ----------
            ffin = []
            for m in range(8):
                ps0 = psB.tile([128, TOK], F32, tag="b", name="ps_f0")
                ps1 = psA.tile([128, 512], F32, tag="av", name="ps_f1")
                for dd in range(2):
                    nc.tensor.matmul(ps0, wi0T[dd][:, m * 128:(m + 1) * 128],
                                     hrT[dd], start=(dd == 0), stop=(dd == 1))
                    nc.tensor.matmul(ps1, wi1T[dd][:, m * 128:(m + 1) * 128],
                                     hrT[dd], start=(dd == 0), stop=(dd == 1))
                g_t = ep.tile([128, TOK], BF16, tag="g", name="g_t")
                nc.scalar.activation(g_t, ps0, gelu_af, bias=b0c[m])
                u_t = ep.tile([128, TOK], BF16, tag="u", name="u_t")
                nc.scalar.activation(u_t, ps1, AF.Identity, bias=b1c[m])
                ft = ppt([128, TOK], BF16, f"ffin{m}")
                nc.vector.tensor_tensor(ft, g_t, u_t, AX.mult)
                ffin.append(ft)
            for pair in range(2):
                ps = psB.tile([128, TOK], F32, tag="b", name="ps_wo")
                for q in range(2):
                    blk = pair * 2 + q
                    for ku in range(8):
                        nc.tensor.matmul(
                            ps[:, q * D:(q + 1) * D],
                            ffin[ku][:, blk * 128:(blk + 1) * 128],
                            woT[ku], start=(ku == 0), stop=(ku == 7))
                for q in range(2):
                    blk = pair * 2 + q
                    o = smp.tile([128, D], F32, tag="o", name="o_sb")
                    nc.vector.tensor_tensor(o, ps[:, q * D:(q + 1) * D],
                                            hid[blk], AX.add)
                    b, s = blk // 2, blk % 2
                    nc.sync.dma_start(
                        out=out_ext[b].rearrange("(s p) d -> s p d", p=128)[s],
                        in_=o)
    nc.finalize()
    return nc


def prepare_in_maps(inputs):
    bf = ml_dtypes.bfloat16
    fp8 = ml_dtypes.float8_e4m3
    f32 = np.float32
    X = np.asarray(inputs["X"], f32)
    Y = np.asarray(inputs["Y"], f32)
    g0 = np.asarray(inputs["g0"], f32)
    b0 = np.asarray(inputs["b0"], f32)
    g1 = np.asarray(inputs["g1"], f32)
    b1 = np.asarray(inputs["b1"], f32)
    Wq0 = np.asarray(inputs["Wq"], f32)
    Wq = Wq0 * g0[None, :]
    bq = np.asarray(inputs["bq"], f32) + Wq0 @ b0
    Wk = np.asarray(inputs["Wk"], f32)
    Wv = np.asarray(inputs["Wv"], f32)
    Wmix = np.asarray(inputs["Wmix"], f32)
    # bk is softmax-shift-invariant -> dropped; bv contributes exactly
    # bv @ Wmix^T to Hid (attention rows sum to 1)
    bqmix = (bq + np.asarray(inputs["bmix"], f32)
             + np.asarray(inputs["bv"], f32) @ Wmix.T)
    wi00 = np.asarray(inputs["wi0"], f32)
    wi10 = np.asarray(inputs["wi1"], f32)
    wi0 = wi00 * g1[None, :]
    wi1 = wi10 * g1[None, :]
    bias0 = wi00 @ b1
    bias1 = wi10 @ b1
    wo = np.asarray(inputs["wo"], f32)

    add_enc = np.asarray(inputs["add_enc"], f32)
    mult_enc = np.asarray(inputs["mult_enc"], f32)
    EF_full = np.exp(add_enc / 16.0) * mult_enc   # [H, q, k]

    def tiles2(w):          # [o, i] -> 2 blocks [128, o] side by side
        return np.ascontiguousarray(w.T).reshape(2, 128, -1)

    w8 = np.concatenate(list(tiles2(Wq)) + list(tiles2(Wk)) + list(tiles2(Wv))
                        + list(tiles2(Wmix)), axis=1)
    assert w8.shape == (128, 2048)
    wbb = np.concatenate(
        [np.concatenate(list(tiles2(wi0)) + list(tiles2(wi1)), axis=1)]
        + [np.ascontiguousarray(wo.T).reshape(8, 128, D).transpose(
            1, 0, 2).reshape(128, 8 * D)]
        + list(tiles2(Wq)),
        axis=1)
    assert wbb.shape == (128, 6656)
    bias = np.zeros((128, 24), np.float32)
    bias[0:64, 0:4] = bq.reshape(4, 64).T
    bias[:, 4:12] = bias0.reshape(8, 128).T
    bias[:, 12:20] = bias1.reshape(8, 128).T
    common = {
        "YTb": np.ascontiguousarray(Y.transpose(0, 2, 1)).reshape(
            4, 128, N).transpose(1, 0, 2).reshape(128, 4 * N).astype(fp8),
        "W8": (w8 * 16.0).astype(fp8),
        "Wb": wbb.astype(bf),
        "bias": bias,
        "bqmix": bqmix,
    }
    in_maps = []
    for c in range(NCORES):
        sl = slice(c * NL, (c + 1) * NL)
        m = dict(common)
        m["Xs"] = np.ascontiguousarray(X[:, sl, :])
        efc = EF_full[:, sl, :].transpose(0, 2, 1)   # [H, k, t]
        m["EF"] = np.ascontiguousarray(
            efc.reshape(H, NKT, 128, NL).transpose(0, 2, 1, 3)).reshape(
            H, 128, NKT * NL).astype(bf)
        in_maps.append(m)
    return in_maps


def kernel(**inputs):
    in_maps = prepare_in_maps(inputs)
    nc = build_kernel()
    res = run_bass_kernel_spmd(nc, in_maps, list(range(NCORES)))
    out = np.empty((B, N, D), np.float32)
    for c in range(NCORES):
        out[:, c * NL:(c + 1) * NL, :] = res.results[c]["out"]
    return out


if __name__ == "__main__":
    nc = build_kernel()
    print("build OK")


# revision 40
# speedup vs baseline: 1.1610x; 1.1610x over previous
"""Trainium2 Bass kernel for nn_MAB_17471926960685 (dense_transformer).

Sharding: token-parallel over N. Each of 8 cores takes a 256-token slice of
N (both batches); K/V are computed replicated from Y. No collectives.

Design:
- Host folds both encoding tables into ONE fused table
  EF = exp(add_enc/16) * mult_enc, streamed bf16 in DMA-optimal layout.
  Softmax denominator approximated by sum_k at (mult_enc perturbs it by
  ~0.05%). bk dropped (softmax shift-invariant), bv folded into the
  mix-stage row bias, LN gains folded into weights.
- Scores keys-major: QK matmuls, ScalarE exp with scale=1/16, EF multiply
  split DVE/GpSimd, AV matmul with [V|ones] stationary so the denominator
  rides as output partition 32; normalize via f32r broadcast matmul.
- fp8 (e4m3, x16-scaled weights) with DoubleRow matmuls on the
  score/attention-value paths only (K, V, Q-scores, mix); the residual Q
  and FFN stay bf16 for accuracy.
- All layouts host-pretransposed; consolidated blob DMAs.
"""

import math
import sys

import numpy as np
import ml_dtypes

sys.path.insert(0, "/opt/trn_rl_repo")

import concourse.bass as bass
import concourse.mybir as mybir
import concourse.tile as tile
from concourse import bacc
from concourse.masks import make_identity
from concourse.bass_utils import run_bass_kernel_spmd

B, N, D, H = 2, 2048, 256, 8
DS = D // H          # 32
NCORES = 8
NL = N // NCORES     # 256 tokens per core per batch
TOK = B * NL         # 512 tokens per core
NKT = N // 128       # 16 key tiles
EPS = 1e-5
F32 = mybir.dt.float32
F32R = mybir.dt.float32r
BF16 = mybir.dt.bfloat16
FP8 = mybir.dt.float8e4
AX = mybir.AluOpType
AF = mybir.ActivationFunctionType
DR = mybir.MatmulPerfMode.DoubleRow

# how many of the 4 EF-mult chunks per (h,b) go to GpSimd (rest on DVE)
POOL_EF_CHUNKS = 0


def build_kernel(gelu_af=AF.Gelu_apprx_tanh):
    nc = bacc.Bacc()
    P = {}
    P["EF"] = nc.declare_dram_parameter("EF", [H, 128, NKT * NL], BF16,
                                        isOutput=False)
    P["W8"] = nc.declare_dram_parameter("W8", [128, 2048], FP8, isOutput=False)
    P["YTb"] = nc.declare_dram_parameter("YTb", [128, 4 * N], FP8,
                                         isOutput=False)
    P["Wb"] = nc.declare_dram_parameter("Wb", [128, 6656], BF16,
                                        isOutput=False)
    for name, shape in [
        ("Xs", [B, NL, D]),
        ("bias", [128, 24]),
        ("bqmix", [D]),
    ]:
        P[name] = nc.declare_dram_parameter(name, shape, F32, isOutput=False)
    out_ext = nc.declare_dram_parameter("out", [B, NL, D], F32, isOutput=True)

    with tile.TileContext(nc) as tc:
        with tc.tile_pool(name="pp", bufs=1) as pp, \
             tc.tile_pool(name="efp", bufs=2) as efp, \
             tc.tile_pool(name="atp", bufs=2) as atp, \
             tc.tile_pool(name="ep", bufs=4) as ep, \
             tc.tile_pool(name="smp", bufs=3) as smp, \
             tc.tile_pool(name="psS", bufs=2, space="PSUM") as psS, \
             tc.tile_pool(name="psA", bufs=2, space="PSUM") as psA, \
             tc.tile_pool(name="psB", bufs=2, space="PSUM") as psB:

            def ppt(shape, dtype, nm):
                return pp.tile(shape, dtype, tag=nm, name=nm)

            # ---------- constants ----------
            id128 = ppt([128, 128], BF16, "id128")
            make_identity(nc, id128)
            eps_t = ppt([128, 1], F32, "eps_t")
            nc.vector.memset(eps_t, EPS)

            # ---------- consolidated input loads ----------
            w8 = ppt([128, 2048], FP8, "w8")
            nc.sync.dma_start(out=w8, in_=P["W8"][:, :])
            ytb = ppt([128, 4 * N], FP8, "ytb")
            nc.scalar.dma_start(out=ytb, in_=P["YTb"][:, :])
            x_ts = []
            for b in range(B):
                x_t = ppt([128, 2 * D], F32, f"x_t{b}")
                nc.sync.dma_start(
                    out=x_t.rearrange("p (s d) -> p s d", s=2),
                    in_=P["Xs"][b].rearrange("(s p) d -> p s d", p=128))
                x_ts.append(x_t)
            bias_t = ppt([128, 24], F32, "bias_t")
            nc.scalar.dma_start(out=bias_t, in_=P["bias"][:, :])
            wb = ppt([128, 6656], BF16, "wb")
            nc.sync.dma_start(out=wb, in_=P["Wb"][:, :])
            # bqmix broadcast row -> [128, D]
            bqmix_bc = ppt([128, D], F32, "bqmix_bc")
            ap = P["bqmix"][:].rearrange("(o d) -> o d", o=1)
            bap = bass.AP(tensor=ap.tensor, offset=ap.offset,
                          ap=[[0, 128], ap.ap[1]])
            nc.sync.dma_start(out=bqmix_bc, in_=bap)

            # fp8 blob slices as [K, 2, M] APs (DoubleRow; weights x16)
            def pair2(ap2d):
                return ap2d.rearrange("p (t c) -> p t c", t=2)

            WqT8 = pair2(w8[:, 0:512])            # [128, 2, 256]
            WkT8 = pair2(w8[:, 512:1024])
            WvT8 = pair2(w8[:, 1024:1536])
            WmixT8 = pair2(w8[:, 1536:2048])
            yT8 = [pair2(ytb[:, b * 2 * N:(b * 2 + 2) * N]) for b in range(B)]
            # bf16 blob slices
            wi0T = [wb[:, dd * 1024:(dd + 1) * 1024] for dd in range(2)]
            wi1T = [wb[:, 2048 + dd * 1024:2048 + (dd + 1) * 1024]
                    for dd in range(2)]
            woT = [wb[:, 4096 + m * D:4096 + (m + 1) * D] for m in range(8)]
            WqTb = [wb[:, 6144 + dd * D:6144 + (dd + 1) * D]
                    for dd in range(2)]
            bqc = [bias_t[0:64, gg:gg + 1] for gg in range(4)]
            b0c = [bias_t[:, 4 + m:5 + m] for m in range(8)]
            b1c = [bias_t[:, 12 + m:13 + m] for m in range(8)]

            # ---------- phase 1: LN0 (DVE; overlaps K/V on PE) ----------
            lnx = []
            for b in range(B):
                x_t = x_ts[b]
                for s in range(2):
                    xa = x_t[:, s * D:(s + 1) * D]
                    stats = smp.tile([128, 6], F32, tag="st", name="stats")
                    mv = smp.tile([128, 2], F32, tag="mv", name="mv")
                    nc.vector.bn_stats(out=stats, in_=xa)
                    nc.vector.bn_aggr(out=mv, in_=stats)
                    std = smp.tile([128, 1], F32, tag="std", name="std")
                    nc.scalar.activation(std, mv[:, 1:2], AF.Sqrt, bias=eps_t)
                    rstd = smp.tile([128, 1], F32, tag="rstd", name="rstd")
                    nc.vector.reciprocal(rstd, std)
                    o = ppt([128, D], BF16, f"lnx{b}{s}")
                    nc.vector.tensor_scalar(o, xa, mv[:, 0:1], rstd,
                                            AX.subtract, AX.mult)
                    lnx.append(o)

            # ---------- phase 2: K^T/V as chunk emitters ----------
            # Emitted partly up front, partly interleaved into the attention
            # loop as PE filler (keeps the PE busy while exp/EF-mult run).
            kT = [ppt([64, N], BF16, f"kT{b}{gg}")
                  for b in range(B) for gg in range(4)]
            vN33 = []
            for b in range(B):
                t = ppt([128, NKT * 264], BF16, f"vN33_{b}")
                nc.gpsimd.memset(
                    t.rearrange("p (k h c) -> p k h c",
                                k=NKT, h=8)[:, :, :, 32:33],
                    1.0)
                vN33.append(t)

            def emit_k_chunk(b, gg, ch):
                t = kT[b * 4 + gg]
                ps = psB.tile([128, TOK], F32, tag="b", name="ps_k")
                sl = slice(ch * 512, (ch + 1) * 512)
                nc.tensor.matmul(
                    ps[0:64, :], WkT8[:, :, gg * 64:(gg + 1) * 64],
                    yT8[b][:, :, sl], start=True, stop=True, perf_mode=DR)
                nc.vector.tensor_scalar(t[:, sl], ps[0:64, :],
                                        1.0 / 16.0, None, AX.mult)

            def emit_v_chunk(b, ktp):
                ps = psB.tile([128, TOK], F32, tag="b", name="ps_v")
                for q in range(2):
                    kt = ktp * 2 + q
                    nc.tensor.matmul(
                        ps[:, q * D:(q + 1) * D],
                        yT8[b][:, :, kt * 128:(kt + 1) * 128],
                        WvT8, start=True, stop=True, perf_mode=DR)
                dst = vN33[b][:, ktp * 528:(ktp + 1) * 528].rearrange(
                    "p (k h c) -> p k h c", k=2, h=8)[:, :, :, 0:32]
                src = ps.rearrange("p (k h c) -> p k h c", k=2, h=8)
                nc.scalar.mul(dst, src, 1.0 / 16.0)

            # up-front: V(b0) + K(b0, gg0)
            for ktp in range(8):
                emit_v_chunk(0, ktp)
            for ch in range(4):
                emit_k_chunk(0, 0, ch)
            # filler queue, deadline-ordered for b-major item order
            fillers = []
            for gg in (1, 2, 3):
                fillers += [lambda gg=gg, ch=ch: emit_k_chunk(0, gg, ch)
                            for ch in range(4)]
            fillers += [lambda ch=ch: emit_k_chunk(1, 0, ch)
                        for ch in range(4)]
            fillers += [lambda ktp=ktp: emit_v_chunk(1, ktp)
                        for ktp in range(8)]
            for gg in (1, 2, 3):
                fillers += [lambda gg=gg, ch=ch: emit_k_chunk(1, gg, ch)
                            for ch in range(4)]
            FILL_PER_STEP = [3, 3, 3, 3, 3, 3, 3, 3, 2, 2, 2, 2, 2, 2, 0, 0]

            # transpose lnx -> lnxT (de-major): bf16 for qN, fp8 for scores
            lnxTb = ppt([128, 2 * TOK], BF16, "lnxTb")
            lnxT8 = ppt([128, 2 * TOK], FP8, "lnxT8")
            for dd in range(2):
                pt = psB.tile([128, TOK], F32, tag="b", name="pt_lnx")
                ptb = pt.bitcast(BF16)          # [128, 1024] bf16 view
                for tt in range(4):
                    nc.tensor.transpose(ptb[:, tt * 128:(tt + 1) * 128],
                                        lnx[tt][:, dd * 128:(dd + 1) * 128],
                                        id128)
                nc.vector.tensor_copy(lnxTb[:, dd * TOK:(dd + 1) * TOK],
                                      ptb[:, 0:TOK])
                nc.vector.tensor_copy(lnxT8[:, dd * TOK:(dd + 1) * TOK],
                                      ptb[:, 0:TOK])
            lnxT8p = pair2(lnxT8[:, :])

            # Q^T (de-major, fp8 DR) for scores: 4 tiles of 64 rows
            # (2 heads each; operand base partitions must be in {0,32,64})
            qsT = []
            for gg in range(4):
                ps = psB.tile([128, TOK], F32, tag="b", name="ps_q")
                nc.tensor.matmul(ps[0:64, :],
                                 WqT8[:, :, gg * 64:(gg + 1) * 64],
                                 lnxT8p, start=True, stop=True, perf_mode=DR)
                t = ppt([64, TOK], BF16, f"qsT{gg}")
                nc.vector.tensor_scalar(t, ps[0:64, :], 1.0 / 16.0, bqc[gg],
                                        AX.mult, AX.add)
                qsT.append(t)
            # Q residual (tok-major, bf16) + bqmix row
            qN = []
            for pair in range(2):
                ps = psB.tile([128, TOK], F32, tag="b", name="ps_qn")
                for q in range(2):
                    blk = pair * 2 + q
                    for dd in range(2):
                        nc.tensor.matmul(
                            ps[:, q * D:(q + 1) * D],
                            lnxTb[:, dd * TOK + blk * 128:
                                  dd * TOK + (blk + 1) * 128],
                            WqTb[dd], start=(dd == 0), stop=(dd == 1))
                for q in range(2):
                    t = ppt([128, D], F32, f"qN{pair * 2 + q}")
                    nc.vector.tensor_tensor(t, ps[:, q * D:(q + 1) * D],
                                            bqmix_bc, AX.add)
                    qN.append(t)

            hid, hr = [None] * 4, [None] * 4

            def emit_ln1(blk):
                stats = smp.tile([128, 6], F32, tag="st", name="stats1")
                mv = smp.tile([128, 2], F32, tag="mv", name="mv1")
                nc.vector.bn_stats(out=stats, in_=hid[blk])
                nc.vector.bn_aggr(out=mv, in_=stats)
                std = smp.tile([128, 1], F32, tag="std", name="std1")
                nc.scalar.activation(std, mv[:, 1:2], AF.Sqrt, bias=eps_t)
                rstd = smp.tile([128, 1], F32, tag="rstd", name="rstd1")
                nc.vector.reciprocal(rstd, std)
                t = ppt([128, D], BF16, f"hr{blk}")
                nc.vector.tensor_scalar(t, hid[blk], mv[:, 0:1], rstd,
                                        AX.subtract, AX.mult)
                hr[blk] = t

            def emit_mix(pair):
                ps = psB.tile([128, TOK], F32, tag="b", name="ps_mx")
                for q in range(2):
                    blk = pair * 2 + q
                    nc.tensor.matmul(
                        ps[:, q * D:(q + 1) * D],
                        mhT8p[:, :, blk * 128:(blk + 1) * 128],
                        WmixT8, start=True, stop=True, perf_mode=DR)
                for q in range(2):
                    blk = pair * 2 + q
                    t = ppt([128, D], F32, f"hid{blk}")
                    nc.vector.scalar_tensor_tensor(
                        t, ps[:, q * D:(q + 1) * D], 1.0 / 256.0, qN[blk],
                        AX.mult, AX.add)
                    hid[blk] = t
                for q in range(2):
                    emit_ln1(pair * 2 + q)

            # ---------- phase 3: attention ----------
            # 2-stage software pipeline: step i emits rcp(i-2),
            # QK/exp/EFmult(i), AV(i-1), bc/normalize(i-2)
            mhT8 = ppt([128, 2 * TOK], FP8, "mhT8")
            mhT8p = pair2(mhT8[:, :])

            def emit_av(item):
                at, h, b = item[0], item[2], item[3]
                psav = psA.tile([128, 512], F32, tag="av", name="psav")
                item[1] = psav
                for kt in range(NKT):
                    nc.tensor.matmul(
                        psav[0:33, 0:256],
                        vN33[b][:, kt * 264 + h * 33:kt * 264 + h * 33 + 33],
                        at[:, kt * 256:(kt + 1) * 256],
                        start=(kt == 0), stop=(kt == NKT - 1))

            def emit_rcp(item):
                psav = item[1]
                rcp = smp.tile([1, 256], F32, tag="rcp", name="rcp")
                item[4] = rcp
                nc.vector.reciprocal(rcp, psav[32:33, 0:256])
                nc.vector.tensor_scalar_mul(rcp, rcp, 16.0)
                rb = smp.tile([32, 256], F32, tag="rb", name="rb")
                item[5] = rb
                nc.gpsimd.partition_broadcast(rb, rcp)

            def emit_final(item):
                psav, h, b, rb = item[1], item[2], item[3], item[5]
                g, j = h // 4, h % 4
                nc.vector.tensor_tensor(
                    mhT8[j * 32:(j + 1) * 32,
                         g * TOK + b * 256:g * TOK + (b + 1) * 256],
                    psav[0:32, 0:256], rb, AX.mult)

            items = []
            fq = list(fillers)
            for b in range(B):
                for h in range(H):
                    gg, jj = h // 2, h % 2
                    i = len(items)
                    ef_t = efp.tile([128, NKT * NL], BF16, tag="ef",
                                    name="ef_t")
                    nc.sync.dma_start(out=ef_t, in_=P["EF"][h])
                    if i >= 2:
                        emit_rcp(items[i - 2])   # DVE: ahead of EF-mults
                    for _ in range(FILL_PER_STEP[i]):
                        if fq:
                            fq.pop(0)()
                    at = atp.tile([128, NKT * NL], BF16, tag="at", name="at")
                    for c in range(4):
                        ps = psS.tile([128, 1024], F32, tag="s", name="ps_s")
                        for q in range(4):
                            kt = c * 4 + q
                            nc.tensor.matmul(
                                ps[:, q * 256:(q + 1) * 256],
                                kT[b * 4 + gg][jj * 32:(jj + 1) * 32,
                                               kt * 128:(kt + 1) * 128],
                                qsT[gg][jj * 32:(jj + 1) * 32,
                                        b * 256:(b + 1) * 256],
                                start=True, stop=True)
                        e_t = ep.tile([128, 1024], BF16, tag="e", name="e_t")
                        nc.scalar.activation(e_t, ps, AF.Exp, scale=1.0 / 16.0)
                        eng = nc.gpsimd if c < POOL_EF_CHUNKS else nc.vector
                        eng.tensor_tensor(
                            at[:, c * 1024:(c + 1) * 1024], e_t,
                            ef_t[:, c * 1024:(c + 1) * 1024], AX.mult)
                    items.append([at, None, h, b, None, None])
                    if i >= 1:
                        emit_av(items[i - 1])
                    if i >= 2:
                        emit_final(items[i - 2])
            assert not fq
            emit_av(items[-1])
            emit_rcp(items[-2])
            emit_final(items[-2])
            emit_rcp(items[-1])
            emit_final(items[-1])

            # ---------- phase 4: mix (fp8 DR) + residual + LN1 ----------
            emit_mix(0)
            emit_mix(1)
            hrT = []
            for dd in range(2):
                pt = psA.tile([128, 512], F32, tag="av", name="pt_hr")
                ptb = pt.bitcast(BF16)
                for tt in range(4):
                    nc.tensor.transpose(ptb[:, tt * 128:(tt + 1) * 128],
                                        hr[tt][:, dd * 128:(dd + 1) * 128],
                                        id128)
                t = ppt([128, TOK], BF16, f"hrT{dd}")
                nc.vector.tensor_copy(t, ptb[:, 0:TOK])
                hrT.append(t)

            # ---------- phase 5: FFN (bf16) ----------
            ffin = []
            for m in range(8):
                ps0 = psB.tile([128, TOK], F32, tag="b", name="ps_f0")
                ps1 = psA.tile([128, 512], F32, tag="av", name="ps_f1")
                for dd in range(2):
                    nc.tensor.matmul(ps0, wi0T[dd][:, m * 128:(m + 1) * 128],
                                     hrT[dd], start=(dd == 0), stop=(dd == 1))
                    nc.tensor.matmul(ps1, wi1T[dd][:, m * 128:(m + 1) * 128],
                                     hrT[dd], start=(dd == 0), stop=(dd == 1))
                g_t = ep.tile([128, TOK], BF16, tag="g", name="g_t")
                nc.scalar.activation(g_t, ps0, gelu_af, bias=b0c[m])
                u_t = ep.tile([128, TOK], BF16, tag="u", name="u_t")
                nc.scalar.activation(u_t, ps1, AF.Identity, bias=b1c[m])
                ft = ppt([128, TOK], BF16, f"ffin{m}")
                nc.vector.tensor_tensor(ft, g_t, u_t, AX.mult)
                ffin.append(ft)
            for pair in range(2):
                ps = psB.tile([128, TOK], F32, tag="b", name="ps_wo")
                for q in range(2):
                    blk = pair * 2 + q
                    for ku in range(8):
                        nc.tensor.matmul(
                            ps[:, q * D:(q + 1) * D],
                            ffin[ku][:, blk * 128:(blk + 1) * 128],
                            woT[ku], start=(ku == 0), stop=(ku == 7))
                for q in range(2):
                    blk = pair * 2 + q
                    o = smp.tile([128, D], F32, tag="o", name="o_sb")
                    nc.vector.tensor_tensor(o, ps[:, q * D:(q + 1) * D],
                                            hid[blk], AX.add)
                    b, s = blk // 2, blk % 2
                    nc.sync.dma_start(
                        out=out_ext[b].rearrange("(s p) d -> s p d", p=128)[s],
                        in_=o)
    nc.finalize()
    return nc


def prepare_in_maps(inputs):
    bf = ml_dtypes.bfloat16
    fp8 = ml_dtypes.float8_e4m3
    f32 = np.float32
    X = np.asarray(inputs["X"], f32)
    Y = np.asarray(inputs["Y"], f32)
    g0 = np.asarray(inputs["g0"], f32)
    b0 = np.asarray(inputs["b0"], f32)
    g1 = np.asarray(inputs["g1"], f32)
    b1 = np.asarray(inputs["b1"], f32)
    Wq0 = np.asarray(inputs["Wq"], f32)
    Wq = Wq0 * g0[None, :]
    bq = np.asarray(inputs["bq"], f32) + Wq0 @ b0
    Wk = np.asarray(inputs["Wk"], f32)
    Wv = np.asarray(inputs["Wv"], f32)
    Wmix = np.asarray(inputs["Wmix"], f32)
    # bk is softmax-shift-invariant -> dropped; bv contributes exactly
    # bv @ Wmix^T to Hid (attention rows sum to 1)
    bqmix = (bq + np.asarray(inputs["bmix"], f32)
             + np.asarray(inputs["bv"], f32) @ Wmix.T)
    wi00 = np.asarray(inputs["wi0"], f32)
    wi10 = np.asarray(inputs["wi1"], f32)
    wi0 = wi00 * g1[None, :]
    wi1 = wi10 * g1[None, :]
    bias0 = wi00 @ b1
    bias1 = wi10 @ b1
    wo = np.asarray(inputs["wo"], f32)

    add_enc = np.asarray(inputs["add_enc"], f32)
    mult_enc = np.asarray(inputs["mult_enc"], f32)
    EF_full = np.exp(add_enc / 16.0) * mult_enc   # [H, q, k]

    def tiles2(w):          # [o, i] -> 2 blocks [128, o] side by side
        return np.ascontiguousarray(w.T).reshape(2, 128, -1)

    w8 = np.concatenate(list(tiles2(Wq)) + list(tiles2(Wk)) + list(tiles2(Wv))
                        + list(tiles2(Wmix)), axis=1)
    assert w8.shape == (128, 2048)
    wbb = np.concatenate(
        [np.concatenate(list(tiles2(wi0)) + list(tiles2(wi1)), axis=1)]
        + [np.ascontiguousarray(wo.T).reshape(8, 128, D).transpose(
            1, 0, 2).reshape(128, 8 * D)]
        + list(tiles2(Wq)),
        axis=1)
    assert wbb.shape == (128, 6656)
    bias = np.zeros((128, 24), np.float32)
    bias[0:64, 0:4] = bq.reshape(4, 64).T
    bias[:, 4:12] = bias0.reshape(8, 128).T
    bias[:, 12:20] = bias1.reshape(8, 128).T
    common = {
        "YTb": np.ascontiguousarray(Y.transpose(0, 2, 1)).reshape(
            4, 128, N).transpose(1, 0, 2).reshape(128, 4 * N).astype(fp8),
        "W8": (w8 * 16.0).astype(fp8),
        "Wb": wbb.astype(bf),
        "bias": bias,
        "bqmix": bqmix,
    }
    in_maps = []
    for c in range(NCORES):
        sl = slice(c * NL, (c + 1) * NL)
        m = dict(common)
        m["Xs"] = np.ascontiguousarray(X[:, sl, :])
        efc = EF_full[:, sl, :].transpose(0, 2, 1)   # [H, k, t]
        m["EF"] = np.ascontiguousarray(
            efc.reshape(H, NKT, 128, NL).transpose(0, 2, 1, 3)).reshape(
            H, 128, NKT * NL).astype(bf)
        in_maps.append(m)
    return in_maps


def kernel(**inputs):
    in_maps = prepare_in_maps(inputs)
    nc = build_kernel()
    res = run_bass_kernel_spmd(nc, in_maps, list(range(NCORES)))
    out = np.empty((B, N, D), np.float32)
    for c in range(NCORES):
        out[:, c * NL:(c + 1) * NL, :] = res.results[c]["out"]
    return out


if __name__ == "__main__":
    nc = build_kernel()
    print("build OK")
